# revision 42
# baseline (speedup 1.0000x reference)
"""Trainium2 Bass kernel for nn_LogicAutoEncoder.

Math: board_state (B,9,3) one-hot -> logits (B,9,3).
  sim[b,r,p,i] depends on the board only through cell state c = state(b,i),
  so sim = T[r,p,i,c] (a 432-entry table, computed on host).  The max over
  i is replaced by a 32-norm:  max_i x_i ~= (sum_i x_i^32)^(1/32), which
  turns the whole reduction into a LINEAR op over the one-hot input:
    S[b,(r,p)] = onehot[b] @ (T/M)^32        (one tiny matmul, no reduce)
    act[b,r]   = (S0*S1)^(1/32)             = exp((ln S0 + ln S1)/32)
    out        = act @ (heads*M0*M1) + bias  (bias via act ones column)
  Measured on-device error: rel_fro ~= 1.0e-2 (gate 2e-2).

Device pipeline (pure data parallel over 8 cores, 65536 rows each) over
units of 1-2 supertiles (4096 rows each; single supertiles at the pipeline
head/tail halve the fill/drain ladder, pairs in the middle); the input is
host-transposed to feature-major (108,1024) fp8 tiles (one-hot is exact in
fp8) so NO input transposes or staging copies are needed:
  1. DMA in (108, 1024n) fp8 (SP HWDGE)
  2. PE: 8n matmuls lhsT=X-chunk (108,128) fp8 @ W2 (108,64 block-diag)
     bf16 -> S PSUM (128,512n) f32   [64-col streams: cheap]
  3. ACT: sat^(1/32) = Exp(ln2/2^23/32 * u32bits(S) - 127 ln2/32) -> bf16
     (bitcast fast-log: ACT converts uint32 input to its integer value, so
     one activation does log+root; mantissa-linearization error ~0.06/32);
     Pool: 2x-mode multiply sat0^(1/32)*sat1^(1/32) -> act (128,32n,9)
     bf16 with persistent ones column (bias folded into heads row 8)
  4. PE: 3n transposes (bf16, 1 cyc/row) -> PSUM bf16; DVE 2x copies -> aT
  5. PE: 3n block-diag heads matmuls (bf16, slice groups 8/10/14) -> PSUM
  6. PSUM->SBUF bf16 out copies split DVE/ACT (po2->DVE, po1->ACT/split)
  7. DMA out (128, 864n) bf16 (SP HWDGE)
Emission is software-pipelined 2 units ahead (early stages before older
units' late stages) to avoid head-of-line blocking in the in-order engine
queues.  Host un-permutes the (st, m, slice, 27) output layout -> f32.
"""

import functools
import os
import sys

import numpy as np

sys.path.insert(0, "/opt/trn_rl_repo")

B = 524288
N_CORES = 8
BC = B // N_CORES            # 65536 rows per core
ST_ROWS = 4096               # rows per supertile
N_ST = BC // ST_ROWS         # 16 supertiles
N_PAIR = N_ST // 2           # DMA pairs
P = 32                       # p-norm exponent
HGRP = [(0, 8), (8, 10), (18, 14)]  # heads-stage slice groups

# packed singles layout: [idm 128 | w2 64 | hb8 216 | hb10 270 | hb14 378]
W2_C0 = 128
HB_C0 = [192, 408, 678]
WPACK_COLS = 1056


def _build_program():
    import concourse.bacc as bacc
    import concourse.mybir as mybir
    import concourse.tile as tile

    f32 = mybir.dt.float32
    bf16 = mybir.dt.bfloat16
    u32 = mybir.dt.uint32
    fp8 = mybir.dt.float8e4
    Exp = mybir.ActivationFunctionType.Exp
    import math
    exp_scale = math.log(2.0) / (P * (1 << 23))
    exp_bias = -127.0 * math.log(2.0) / P

    nc = bacc.Bacc(
        "TRN2", target_bir_lowering=False, debug=False, num_devices=N_CORES
    )
    x_d = nc.dram_tensor("x", [N_ST * 108, 1024], fp8, kind="ExternalInput")
    x0_d = nc.dram_tensor("x0", [108, 1152], fp8, kind="ExternalInput")
    wp_d = nc.dram_tensor("wp", [128, WPACK_COLS], bf16, kind="ExternalInput")
    out_d = nc.dram_tensor("out", [N_ST * 128, 864], bf16, kind="ExternalOutput")

    x_pairs = x_d.rearrange("(t two p) n -> t p two n", two=2, p=108)
    out_pairs = out_d.rearrange("(t two p) f -> t p two f", two=2, p=128)

    with tile.TileContext(nc) as tc:
        with (
            tc.tile_pool(name="singles", bufs=1) as singles,
            tc.tile_pool(name="xp", bufs=5) as xp_pool,
            tc.tile_pool(name="sr", bufs=4) as sr_pool,
            tc.tile_pool(name="aT", bufs=3) as aT_pool,
            tc.tile_pool(name="ob", bufs=3) as ob_pool,
            tc.tile_pool(name="p_S", bufs=2, space="PSUM") as pS_pool,
            tc.tile_pool(name="p_pa", bufs=1, space="PSUM") as pa_pool,
            tc.tile_pool(name="p_po1", bufs=2, space="PSUM") as po1_pool,
            tc.tile_pool(name="p_po2", bufs=1, space="PSUM") as po2_pool,
        ):
            # unit-0 input tile also carries w2 (bf16 bytes bitcast to fp8)
            # so the first matmuls wait on a single DMA chain
            x0_sb = singles.tile([108, 1152], fp8)
            nc.sync.dma_start(out=x0_sb[:], in_=x0_d[:])
            w2 = x0_sb[:, 1024:1152].bitcast(bf16)
            wp_sb = singles.tile([128, WPACK_COLS], bf16)
            idm = wp_sb[:, 0:128]
            hbs = [
                wp_sb[0 : ns * 9, HB_C0[gi] : HB_C0[gi] + ns * 27]
                for gi, (s0, ns) in enumerate(HGRP)
            ]

            act_bufs = [
                singles.tile([128, 64, 9], bf16, name=f"act{i}") for i in range(3)
            ]
            for ab in act_bufs:
                nc.gpsimd.memset(ab[:, :, 8:9], 1.0)
            ebias = singles.tile([128, 1], f32)
            nc.gpsimd.memset(ebias[:], exp_bias)
            # preload the Exp activation table before the pipeline needs it
            scr = singles.tile([128, 1], f32)
            nc.scalar.activation(scr[:], ebias[:], Exp)

            # Units: single supertiles at pipeline head/tail (halved ladder
            # latency while engines are idle anyway), pairs in the middle.
            # Each unit is (first_supertile, n_supertiles).
            ucfg = os.environ.get("KERNEL_UNITS", "2h2t")
            if ucfg == "pairs":
                UNITS = [(s, 2) for s in range(0, 16, 2)]
            elif ucfg == "2h2t":
                UNITS = [(0, 1), (1, 1)] + [
                    (s, 2) for s in range(2, 14, 2)
                ] + [(14, 1), (15, 1)]
            elif ucfg == "4h0t":
                UNITS = [(0, 1), (1, 1), (2, 1), (3, 1)] + [
                    (s, 2) for s in range(4, 16, 2)
                ]
            elif ucfg == "2h0t":
                UNITS = [(0, 1), (1, 1)] + [(s, 2) for s in range(2, 16, 2)]
            elif ucfg == "3h3t":
                UNITS = [(0, 1), (1, 1), (2, 1)] + [
                    (s, 2) for s in range(3, 13, 2)
                ] + [(13, 1), (14, 1), (15, 1)]
            elif ucfg == "6h2t":
                UNITS = [(s, 1) for s in range(6)] + [
                    (s, 2) for s in range(6, 14, 2)
                ] + [(14, 1), (15, 1)]
            else:  # 4h2t
                UNITS = [(0, 1), (1, 1), (2, 1), (3, 1)] + [
                    (s, 2) for s in range(4, 14, 2)
                ] + [(14, 1), (15, 1)]
            NU = len(UNITS)

            x_sts = x_d.rearrange("(s p) n -> s p n", p=108)
            out_sts = out_d.rearrange("(s p) f -> s p f", p=128)

            x_tiles = [None] * NU

            def dma_in(u):
                s0, n = UNITS[u]
                if u == 0:
                    x_tiles[0] = x0_sb[:, 0:1024]
                    return
                x_tiles[u] = xp_pool.tile([108, 1024 * n], fp8, name="xt", tag="xt")
                if n == 1:
                    nc.sync.dma_start(out=x_tiles[u][:], in_=x_sts[s0])
                else:
                    xv = x_tiles[u][:].rearrange("p (two n) -> p two n", two=2)
                    nc.sync.dma_start(out=xv, in_=x_pairs[s0 // 2])

            def stage_early(u):
                s0, n = UNITS[u]
                xt = x_tiles[u]
                # 8 matmuls per supertile -> S (128, 512n) f32 PSUM
                Sp = pS_pool.tile([128, 512 * n], f32, name="Sp", tag="Sp")
                for half in range(n):
                    for g in range(8):
                        nc.tensor.matmul(
                            Sp[:, half * 512 + g * 64 : half * 512 + (g + 1) * 64],
                            xt[:, half * 1024 + g * 128 : half * 1024 + (g + 1) * 128],
                            w2,
                            start=True,
                            stop=True,
                        )
                # bitcast fast-log, entirely inside one ACT op:
                # sat^(1/32) = exp(ln2/2^23/32 * u32bits(S) - 127*ln2/32)
                # (the ACT engine converts uint32 input to its integer value
                # before the affine + exp).  Then one 2x-mode DVE multiply
                # forms act = sat0^(1/32) * sat1^(1/32) straight into the
                # act tile (ones column pre-set for the bias trick).
                sr_t = sr_pool.tile([128, 512 * n], bf16, name="sr", tag="sr")
                if os.environ.get("KERNEL_EXPSPLIT", "0") == "1" and n == 2:
                    for h in range(2):
                        nc.scalar.activation(
                            sr_t[:, h * 512 : (h + 1) * 512],
                            Sp[:, h * 512 : (h + 1) * 512].bitcast(u32),
                            Exp, scale=exp_scale, bias=ebias[:],
                        )
                else:
                    nc.scalar.activation(
                        sr_t[:], Sp[:].bitcast(u32), Exp,
                        scale=exp_scale, bias=ebias[:],
                    )
                srv = sr_t[:].rearrange("m (ga p r) -> m ga p r", p=2, r=8)
                act = act_bufs[u % 3][:, 0 : 32 * n, :]
                if os.environ.get("KERNEL_MULT", "pool") == "pool":
                    nc.gpsimd.tensor_mul(
                        act[:, :, 0:8], srv[:, :, 0, :], srv[:, :, 1, :]
                    )
                else:
                    nc.vector.tensor_mul(
                        act[:, :, 0:8], srv[:, :, 0, :], srv[:, :, 1, :]
                    )
                x_tiles[u] = None

            def stage_late(u):
                s0, n = UNITS[u]
                act = act_bufs[u % 3][:, 0 : 32 * n, :]

                # transposes -> aT (bf16 PSUM, DVE copy out)
                act2 = act.rearrange("m sl r -> m (sl r)")
                pa = pa_pool.tile([126, 384 * n], bf16, name="pa", tag="pa")
                for half in range(n):
                    for gi, (g0, ns) in enumerate(HGRP):
                        nc.tensor.transpose(
                            pa[
                                0 : ns * 9,
                                half * 384 + gi * 128 : half * 384 + (gi + 1) * 128,
                            ],
                            act2[:, half * 288 + g0 * 9 : half * 288 + (g0 + ns) * 9],
                            idm,
                        )
                aT_t = aT_pool.tile([126, 384 * n], bf16, name="aT", tag="aT")
                for half in range(n):
                    nc.vector.tensor_copy(
                        aT_t[:, half * 384 : (half + 1) * 384],
                        pa[:, half * 384 : (half + 1) * 384],
                    )

                # heads matmuls + PSUM->SBUF bf16 out copies
                ob = ob_pool.tile([128, 864 * n], bf16, name="ob", tag="ob")
                for half in range(n):
                    po1 = po1_pool.tile([128, 486], f32, name="po1", tag="po1")
                    po2 = po2_pool.tile([128, 378], f32, name="po2", tag="po2")
                    for gi, (g0, ns) in [(2, HGRP[2]), (0, HGRP[0]), (1, HGRP[1])]:
                        dst, c0 = (po1, g0 * 27) if gi < 2 else (po2, 0)
                        nc.tensor.matmul(
                            dst[:, c0 : c0 + ns * 27],
                            aT_t[
                                0 : ns * 9,
                                half * 384 + gi * 128 : half * 384 + (gi + 1) * 128,
                            ],
                            hbs[gi],
                            start=True,
                            stop=True,
                        )
                    ocol = half * 864
                    pocfg = os.environ.get("KERNEL_PO", "DDAS")
                    # chars: [po2c_h0, po2c_h1, po1c_h0, po1c_h1] A=ACT D=DVE
                    # S = split po1c: [0:324]->ACT, [324:486]->DVE
                    def _pcopy(eng, dst, srcv):
                        if eng == "A":
                            nc.scalar.copy(dst, srcv)
                        else:
                            nc.vector.tensor_copy(dst, srcv)
                    _pcopy(pocfg[half], ob[:, ocol + 486 : ocol + 864], po2[:])
                    if pocfg[2 + half] == "S":
                        nc.scalar.copy(ob[:, ocol : ocol + 324], po1[:, 0:324])
                        nc.vector.tensor_copy(
                            ob[:, ocol + 324 : ocol + 486], po1[:, 324:486]
                        )
                    else:
                        _pcopy(pocfg[2 + half], ob[:, ocol : ocol + 486], po1[:])

                if n == 1:
                    nc.sync.dma_start(out=out_sts[s0], in_=ob[:])
                else:
                    obv = ob[:].rearrange("p (two f) -> p two f", two=2)
                    nc.sync.dma_start(out=out_pairs[s0 // 2], in_=obv)

            RA = int(os.environ.get("KERNEL_RA", "2"))
            PF = int(os.environ.get("KERNEL_PF", "3"))
            for u in range(min(PF, NU)):
                dma_in(u)
            # remaining weights (identity for transposes, heads blocks)
            nc.sync.dma_start(out=wp_sb[:], in_=wp_d[:])
            for u in range(RA):
                stage_early(u)
            for u in range(NU):
                if u + RA < NU:
                    stage_early(u + RA)
                stage_late(u)
                if u + PF < NU:
                    dma_in(u + PF)

    nc.compile()
    return nc


@functools.cache
def _get_program():
    return _build_program()


def _host_tables(premises, heads, bias):
    """Tiny host-side tables: (T/M)^P block-diag + heads with M folded in."""
    pos = (np.arange(9, dtype=np.float64) - 4.0) / 4.0
    pl = np.array([0.0, 1.0, -1.0], dtype=np.float64)
    prem = premises.astype(np.float64)
    d_pl = (pl[None, None, :] - prem[:, :, 0][:, :, None]) ** 2  # (8,2,3)
    d_pos = (pos[None, None, :] - prem[:, :, 1][:, :, None]) ** 2  # (8,2,9)
    T = np.exp(-(d_pl[:, :, None, :] + d_pos[:, :, :, None]))  # (8,2,9,3)

    M = T.max(axis=(2, 3))  # (8,2)
    Tn = (T / M[:, :, None, None]) ** P
    wtab = Tn.transpose(2, 3, 1, 0).reshape(27, 16)  # [(i,c), (p8, r)]
    wtab = np.where(np.abs(wtab) < 1.18e-38, 0.0, wtab).astype(np.float32)
    w2 = np.zeros((108, 64), dtype=np.float32)
    for a in range(4):
        w2[a * 27 : (a + 1) * 27, a * 16 : (a + 1) * 16] = wtab

    MM = M[:, 0] * M[:, 1]  # (8,)
    h9 = np.zeros((9, 27), dtype=np.float64)
    h9[0:8] = heads.astype(np.float64) * MM[:, None]
    h9[8] = bias.astype(np.float64)
    hbs = []
    for s0, ns in HGRP:
        hb = np.zeros((ns * 9, ns * 27), dtype=np.float32)
        for v in range(ns):
            hb[v * 9 : (v + 1) * 9, v * 27 : (v + 1) * 27] = h9
        hbs.append(hb)
    return w2, hbs


def kernel(board_state, premises, heads, bias):
    import ml_dtypes
    from concourse.bass_utils import run_bass_kernel_spmd

    bf = ml_dtypes.bfloat16
    nc = _get_program()
    w2, hbs = _host_tables(
        np.asarray(premises), np.asarray(heads), np.asarray(bias)
    )
    wpack = np.zeros((128, WPACK_COLS), dtype=np.float32)
    wpack[0:128, 0:128] = np.eye(128, dtype=np.float32)
    wpack[0:108, W2_C0 : W2_C0 + 64] = w2
    for gi, (s0, ns) in enumerate(HGRP):
        wpack[0 : ns * 9, HB_C0[gi] : HB_C0[gi] + ns * 27] = hbs[gi]
    wpack = wpack.astype(bf)

    # host-transpose input to feature-major supertile tiles:
    # x[st*108 + a*27 + f, g*128 + m] = bs[st*4096 + g*512 + a*128 + m, f]
    bs = np.asarray(board_state, dtype=np.float32).reshape(
        N_CORES, N_ST, 8, 4, 128, 27
    )
    x_all = np.ascontiguousarray(bs.transpose(0, 1, 3, 5, 2, 4)).astype(
        ml_dtypes.float8_e4m3fn
    )
    x_all = x_all.reshape(N_CORES, N_ST * 108, 1024)

    # unit-0 tile carries w2 as raw bytes (bf16 pair -> two fp8 slots)
    w2_bytes = np.ascontiguousarray(w2.astype(bf)).view(ml_dtypes.float8_e4m3fn)
    x0s = [
        np.ascontiguousarray(
            np.concatenate([x_all[k][0:108, :], w2_bytes], axis=1)
        )
        for k in range(N_CORES)
    ]
    in_maps = [
        {"x": x_all[k], "x0": x0s[k], "wp": wpack} for k in range(N_CORES)
    ]
    res = run_bass_kernel_spmd(
        nc,
        in_maps,
        core_ids=list(range(N_CORES)),
        trace=bool(int(os.environ.get("KERNEL_TRACE", "0"))),
    )
    # out[st*128 + m, (g*4+a)*27 + o] -> row st*4096 + g*512 + a*128 + m
    outs = [
        np.asarray(r["out"])
        .astype(np.float32)
        .reshape(N_ST, 128, 8, 4, 27)
        .transpose(0, 2, 3, 1, 4)
        .reshape(BC, 27)
        for r in res.results
    ]
    out = np.concatenate(outs, axis=0)
    kernel.last_results = res
    return out.reshape(B, 9, 3)

